# revision 1
# baseline (speedup 1.0000x reference)
"""Low-dim-QK multi-head attention TRN2 Bass kernel v3 (8 NeuronCores).

v3 = v2 (Wtil refactor: out = sum_h A_h @ (V @ Wo_h.T), (batch, head-group)
sharding, bf16 operands, host-side partial-sum reduction) plus the per-matmul
overhead attack:

- walrus `--enable-ldw-opt=true` (redundant load-weight elimination; safe
  here: every stationary operand is bf16 -- the known breakage is fp32/fp32r
  standalone-LDW pairing).
- Loop orders put identical stationary operands on consecutive matmuls:
    Wtil:  lhsT = vT[dt,kt] reused 4x over (h, ob)
    apply: lhsT = Wtil[h,kt,ob] reused 4x over q-chunks (out.T layout)
    scores: lhsT = klT[h,kt] reused 4x over q-chunks (as in v2)
- Denominators via ones-column stationary (1-column LDW is ~free):
  D[1, q] accumulated over kt per 512-chunk, reciprocal, then broadcast to
  rep[128, q] with a ones-row matmul; P is pre-normalized in place on
  VectorE.  Apply chains then accumulate both heads in one PSUM chain (18
  matmuls) and drain with a plain ScalarE copy.
- Output is written as out.T (1024, 2048); the host transposes during the
  partial-sum reduction.
"""

import numpy as np
import ml_dtypes

import concourse.bass as bass
import concourse.mybir as mybir
import concourse.tile as tile
from concourse import bacc
import concourse.bass_utils as _bass_utils
from concourse.bass_utils import run_bass_kernel_spmd

F32 = mybir.dt.float32
BF16 = mybir.dt.bfloat16
AF = mybir.ActivationFunctionType
NP_BF16 = ml_dtypes.bfloat16

# --- redundant-LDW elimination (bass-level post-pass) ---------------------
# The tile scheduler splits every matmul into an explicit Ldweights+Matmult
# pair.  When consecutive matmuls on the PE stream share an identical
# stationary operand, the re-load is redundant: the PE array still holds the
# weights.  walrus' own `--enable-ldw-opt` pass crashes on pre-split pairs,
# so dedup here instead, before nc.compile().  A dropped re-load's RAW wait
# is subsumed by the kept (identical) load's wait - semaphores are monotonic
# counters - but any waits/updates are conservatively moved to the following
# matmul anyway (generate_event_semaphores later legalizes multi-waits).


def _ldw_key(inst):
    ap = inst.ins[0]
    return (
        str(getattr(ap, "memref", None) or getattr(ap, "memsetref", "?")),
        getattr(ap, "offset", None),
        str(getattr(ap, "ap", None)),
        str(getattr(ap, "dtype", None)),
        tuple(inst.tile_position or ()) if getattr(inst, "tile_position", None) is not None else None,
        tuple(inst.tile_size or ()) if getattr(inst, "tile_size", None) is not None else None,
    )


def _dedup_ldweights(nc):
    removed = 0
    for fn in nc.m.functions:
        for blk in fn.blocks:
            insts = blk.instructions
            last_key = None
            keep = []
            i = 0
            n = len(insts)
            while i < n:
                inst = insts[i]
                if getattr(inst, "engine", None) != mybir.EngineType.PE:
                    keep.append(inst)
                    i += 1
                    continue
                op = type(inst).__name__
                if op == "InstLdweights" and getattr(inst, "perf_mode", None) is None:
                    key = _ldw_key(inst)
                    if key == last_key and key is not None:
                        # find the following PE instruction (its matmul) to
                        # inherit this load's waits/updates
                        si = inst.sync_info
                        if si is not None and (si.on_wait or si.on_update):
                            j = i + 1
                            tgt = None
                            while j < n:
                                if getattr(insts[j], "engine", None) == mybir.EngineType.PE:
                                    tgt = insts[j]
                                    break
                                j += 1
                            if tgt is None:
                                keep.append(inst)
                                last_key = key
                                i += 1
                                continue
                            tsi = tgt.sync_info
                            if tsi is None:
                                tgt.sync_info = si
                            else:
                                tsi.on_wait.extend(si.on_wait)
                                tsi.on_update.extend(si.on_update)
                        removed += 1
                        i += 1
                        continue
                    last_key = key
                    keep.append(inst)
                    i += 1
                    continue
                if op == "InstMatmult":
                    keep.append(inst)
                    i += 1
                    continue
                # any other PE instruction invalidates the loaded weights
                last_key = None
                keep.append(inst)
                i += 1
            if len(keep) != n:
                try:
                    insts[:] = keep
                except TypeError:
                    blk.instructions = keep
    return removed
# --------------------------------------------------------------------------

# Note: LDW dedup is numerically UNSAFE on this hardware - walrus pairs each
# MMUL with its LDW via auto-flipping double-buffered weights, so an MMUL
# without its own LDW flips to a stale buffer (verified: rel-err 0.27 with
# dedup, 4.3e-3 without; runtime identical, i.e. the loads were already
# hidden).  Keep False.
DEDUP_LDW = False

B = 2
TQ = 2048
TK = 2048
D = 1024
H = 8
HD = 32
DV = 1024
O = 1024
NCORES = 8
NHG = 4
HPC = H // NHG    # 2 heads per core
DT = D // 128
QB = TQ // 128
SCALE = 1.0 / float(np.sqrt(HD))
MASK_BIAS = -60.0


def _chunks(n, c):
    out = []
    k0 = 0
    while k0 < n:
        out.append((k0, min(c, n - k0)))
        k0 += c
    return out


def _body(nc, t, pools, tkp, ablate=frozenset()):
    qT, kT, vT, wqT, wkT, woT, bq, bk, mb, ones_d, ones_row_d, out = t
    const, stream, stage, ps, ps_d = pools
    KT = tkp // 128

    # ---- persistent SBUF tiles --------------------------------------
    vT_sb = const.tile([128, tkp // 128, DT * 128], BF16, tag="vt")
    woT_sb = const.tile([128, HPC * DT, O], BF16, tag="wo")
    qlT = const.tile([64, TQ], BF16, tag="qlT")
    klT = const.tile([64, tkp], BF16, tag="klT")
    p_sb = [
        const.tile([128, KT, TQ], BF16, tag=f"p{h}", name=f"p{h}")
        for h in range(HPC)
    ]
    w_sb = const.tile([128, HPC * KT, O], BF16, tag="wt")
    dinv = const.tile([1, HPC, TQ], BF16, tag="dinv")
    rep_sb = const.tile([128, HPC, TQ], BF16, tag="rep")
    mb_sb = const.tile([128, KT], F32, tag="mb")
    bq_sb = const.tile([64, 1], F32, tag="bq")
    bk_sb = const.tile([64, 1], F32, tag="bk")
    wq_sb = const.tile([128, DT, 64], BF16, tag="wq")
    wk_sb = const.tile([128, DT, 64], BF16, tag="wk")
    ones_col = const.tile([128, 1], BF16, tag="onc")
    ones_row = const.tile([1, 128], BF16, tag="onr")

    # ---- phase A: projections ---------------------------------------
    # Wtil feeds (6.25 MB) go on the Activation HWDGE queue so they stream in
    # parallel with the SP queue's projection feeds; both overlap proj+scores
    # compute.  Issue first: queue order only matters within a queue.
    if "bigdma" not in ablate:
        for wc in range(DT):
            nc.scalar.dma_start(
                out=woT_sb[:, wc, :],
                in_=woT.ap()[:, wc * O : (wc + 1) * O],
            )
        for kt_ in range(KT):
            nc.scalar.dma_start(
                out=vT_sb[:, kt_, :],
                in_=vT.ap()[:, kt_ * DT * 128 : (kt_ + 1) * DT * 128],
            )
        for wc in range(DT, HPC * DT):
            nc.scalar.dma_start(
                out=woT_sb[:, wc, :],
                in_=woT.ap()[:, wc * O : (wc + 1) * O],
            )
    nc.sync.dma_start(out=wq_sb, in_=wqT.ap().rearrange("(dt p) r -> p dt r", p=128))
    nc.sync.dma_start(out=bq_sb, in_=bq.ap().rearrange("(g p) -> p g", p=64))

    ql_ps = [ps.tile([128, 512], F32, tag="ps", name=f"qlps{i}") for i in range(2)]
    for dt_ in range(DT):
        qc = stream.tile([128, TQ], BF16, tag="qs", name=f"qc{dt_}")
        nc.sync.dma_start(out=qc, in_=qT.ap()[dt_ * 128 : (dt_ + 1) * 128, :])
        if dt_ == 0:
            nc.sync.dma_start(
                out=wk_sb, in_=wkT.ap().rearrange("(dt p) r -> p dt r", p=128)
            )
            nc.sync.dma_start(out=bk_sb, in_=bk.ap().rearrange("(g p) -> p g", p=64))
            nc.sync.dma_start(out=mb_sb, in_=mb.ap())
            nc.sync.dma_start(out=ones_col, in_=ones_d.ap())
            nc.sync.dma_start(out=ones_row, in_=ones_row_d.ap())
        for qc4 in range(4):
            nc.tensor.matmul(
                ql_ps[qc4 // 2][qc4 % 2 * 64 : qc4 % 2 * 64 + 64, :],
                lhsT=wq_sb[:, dt_, :],
                rhs=qc[:, qc4 * 512 : (qc4 + 1) * 512],
                start=(dt_ == 0),
                stop=(dt_ == DT - 1),
            )
    for qc4 in range(4):
        nc.scalar.activation(
            out=qlT[:, qc4 * 512 : (qc4 + 1) * 512],
            in_=ql_ps[qc4 // 2][qc4 % 2 * 64 : qc4 % 2 * 64 + 64, :],
            func=AF.Identity, bias=bq_sb, scale=1.0,
        )

    kchunks = _chunks(tkp, 512)
    kl_ps = [
        ps.tile([128, 512], F32, tag="ps", name=f"klps{i}")
        for i in range((len(kchunks) + 1) // 2)
    ]
    for dt_ in range(DT):
        kc = stream.tile([128, tkp], BF16, tag="ks", name=f"kc{dt_}")
        nc.sync.dma_start(out=kc, in_=kT.ap()[dt_ * 128 : (dt_ + 1) * 128, :])
        for ci, (k0, csz) in enumerate(kchunks):
            nc.tensor.matmul(
                kl_ps[ci // 2][ci % 2 * 64 : ci % 2 * 64 + 64, 0:csz],
                lhsT=wk_sb[:, dt_, :],
                rhs=kc[:, k0 : k0 + csz],
                start=(dt_ == 0),
                stop=(dt_ == DT - 1),
            )
    for ci, (k0, csz) in enumerate(kchunks):
        nc.scalar.activation(
            out=klT[:, k0 : k0 + csz],
            in_=kl_ps[ci // 2][ci % 2 * 64 : ci % 2 * 64 + 64, 0:csz],
            func=AF.Identity, bias=bk_sb, scale=1.0,
        )

    # ---- phase B: scores (heads paired on PE row-strips) ------------
    def emit_scores():
        for kt_ in range(KT):
            for h in range(HPC):
                for qc4 in range(4):
                    st = ps.tile(
                        [128, 512], F32, tag="ps", name=f"st{h}_{kt_}_{qc4}"
                    )
                    nc.tensor.matmul(
                        st,
                        lhsT=klT[h * 32 : h * 32 + 32, kt_ * 128 : (kt_ + 1) * 128],
                        rhs=qlT[h * 32 : h * 32 + 32, qc4 * 512 : (qc4 + 1) * 512],
                        start=True,
                        stop=True,
                    )
                    nc.scalar.activation(
                        out=p_sb[h][:, kt_, qc4 * 512 : (qc4 + 1) * 512],
                        in_=st, func=AF.Exp,
                        bias=mb_sb[:, kt_ : kt_ + 1], scale=SCALE,
                    )

    # ---- phase C: Wtil per (h, kt); h0 only needs woT groups 0..DT-1 and
    # vT k-slice kt, so chains start as soon as those DMAs land -----------
    def emit_wtil_h_kt(h, kt_):
        wps = [
            ps.tile([128, 512], F32, tag="ps", name=f"wp{h}_{kt_}_{i}")
            for i in range(2)
        ]
        for dt_ in range(DT):
            for ob in range(2):
                nc.tensor.matmul(
                    wps[ob],
                    lhsT=vT_sb[:, kt_, dt_ * 128 : (dt_ + 1) * 128],
                    rhs=woT_sb[:, h * DT + dt_, ob * 512 : (ob + 1) * 512],
                    start=(dt_ == 0),
                    stop=(dt_ == DT - 1),
                )
        for ob in range(2):
            nc.vector.tensor_copy(
                out=w_sb[:, h * KT + kt_, ob * 512 : (ob + 1) * 512],
                in_=wps[ob],
            )

    # ---- phase D: denominators + P normalization --------------------
    def emit_denom(h):
        for qc4 in range(4):
            dps = ps_d.tile([1, 512], F32, tag="pd", name=f"dps{h}_{qc4}")
            for kt_ in range(KT):
                nc.tensor.matmul(
                    dps,
                    lhsT=ones_col,
                    rhs=p_sb[h][:, kt_, qc4 * 512 : (qc4 + 1) * 512],
                    start=(kt_ == 0),
                    stop=(kt_ == KT - 1),
                )
            nc.vector.reciprocal(
                out=dinv[:, h, qc4 * 512 : (qc4 + 1) * 512], in_=dps
            )
        for qc4 in range(4):
            rps = ps.tile([128, 512], F32, tag="ps", name=f"rps{h}_{qc4}")
            nc.tensor.matmul(
                rps,
                lhsT=ones_row,
                rhs=dinv[:, h, qc4 * 512 : (qc4 + 1) * 512],
                start=True,
                stop=True,
            )
            nc.vector.tensor_copy(
                out=rep_sb[:, h, qc4 * 512 : (qc4 + 1) * 512], in_=rps
            )
        for kt_ in range(KT):
            nc.vector.tensor_mul(
                p_sb[h][:, kt_, :], p_sb[h][:, kt_, :], rep_sb[:, h, :]
            )

    if "scores" in ablate:
        for h in range(HPC):
            nc.vector.memset(p_sb[h][:, 0, 0:512], 1.0)
    if "wtil" in ablate:
        nc.vector.memset(w_sb[:, 0, 0:512], 1.0)
    if "scores" not in ablate:
        emit_scores()
    # Interleave denominator/normalization per head into the Wtil stream so
    # P-hat for both heads is ready the moment Wtil finishes (no PE bubble
    # before apply).  D_h is emitted once exp_h has certainly drained
    # (ScalarE exp of head h completes well inside the first few Wtil
    # kt-blocks' PE time).
    for h in range(HPC):
        for kt_ in range(KT):
            if "wtil" not in ablate:
                emit_wtil_h_kt(h, kt_)
            if kt_ == min(4, KT - 1) and "denom" not in ablate:
                emit_denom(h)

    # ---- phase E: apply, stationary Wtil[h,kt,ob] reused over qc ----
    if "apply" in ablate:
        return
    out_r = out.ap().rearrange("(ob p) q -> p ob q", p=128)
    for ob in range(8):
        uacc = [
            ps.tile([128, 512], F32, tag="ps", name=f"ua{ob}_{qc4}")
            for qc4 in range(4)
        ]
        for h in range(HPC):
            for kt_ in range(KT):
                for qc4 in range(4):
                    nc.tensor.matmul(
                        uacc[qc4],
                        lhsT=w_sb[:, h * KT + kt_, ob * 128 : (ob + 1) * 128],
                        rhs=p_sb[h][:, kt_, qc4 * 512 : (qc4 + 1) * 512],
                        start=(h == 0 and kt_ == 0),
                        stop=(h == HPC - 1 and kt_ == KT - 1),
                    )
        for qc4 in range(4):
            ot = stage.tile([128, 512], F32, tag="ot", name=f"ot{ob}_{qc4}")
            nc.scalar.copy(out=ot, in_=uacc[qc4])
            nc.sync.dma_start(
                out=out_r[:, ob, qc4 * 512 : (qc4 + 1) * 512], in_=ot
            )


def _build_kernel(nc, t, tkp, loop_n=None, ablate=frozenset()):
    tc = tile.TileContext(nc)
    with tc, nc.allow_low_precision(
        reason="bf16 matmul operands; all accumulation is fp32 in PSUM"
    ):
        pools = (
            tc.alloc_tile_pool(name="const", bufs=1),
            tc.alloc_tile_pool(name="stream", bufs=2),
            tc.alloc_tile_pool(name="stage", bufs=2),
            tc.alloc_tile_pool(name="ps", bufs=6, space="PSUM"),
            tc.alloc_tile_pool(name="ps_d", bufs=2, space="PSUM"),
        )
        if loop_n is None:
            _body(nc, t, pools, tkp, ablate=ablate)
        else:
            with tc.For_i(0, loop_n, 1):
                _body(nc, t, pools, tkp, ablate=ablate)
        for p in reversed(pools):
            p.release()
    if DEDUP_LDW:
        _dedup_ldweights(nc)
    nc.compile()


def _declare_io(nc, tkp):
    qT = nc.dram_tensor("qT", (D, TQ), BF16, kind="ExternalInput")
    kT = nc.dram_tensor("kT", (D, tkp), BF16, kind="ExternalInput")
    vT = nc.dram_tensor("vT", (128, DT * tkp), BF16, kind="ExternalInput")  # k-sliced
    wqT = nc.dram_tensor("wqT", (D, 64), BF16, kind="ExternalInput")
    wkT = nc.dram_tensor("wkT", (D, 64), BF16, kind="ExternalInput")
    woT = nc.dram_tensor("woT", (128, HPC * DT * O), BF16, kind="ExternalInput")
    bq = nc.dram_tensor("bq", (64,), F32, kind="ExternalInput")
    bk = nc.dram_tensor("bk", (64,), F32, kind="ExternalInput")
    mb = nc.dram_tensor("mb", (128, tkp // 128), F32, kind="ExternalInput")
    ones_d = nc.dram_tensor("ones_d", (128, 1), BF16, kind="ExternalInput")
    ones_row_d = nc.dram_tensor("ones_row_d", (1, 128), BF16, kind="ExternalInput")
    out = nc.dram_tensor("out", (O, TQ), F32, kind="ExternalOutput")
    return (qT, kT, vT, wqT, wkT, woT, bq, bk, mb, ones_d, ones_row_d, out)


def build_nc(tkp=1152, loop_n=None, ablate=frozenset()):
    nc = bacc.Bacc("TRN2", target_bir_lowering=False, debug=False,
                   num_devices=NCORES)
    t = _declare_io(nc, tkp)
    _build_kernel(nc, t, tkp, loop_n=loop_n, ablate=ablate)
    return nc


_NC_CACHE = {}


def _get_nc(tkp):
    if tkp not in _NC_CACHE:
        _NC_CACHE[tkp] = build_nc(tkp)
    return _NC_CACHE[tkp]


def _prep_in_maps(Q, K, V, Wq, bq, Wk, bk, Wo, bo, key_padding_mask):
    Q = np.asarray(Q, dtype=np.float32)
    K = np.asarray(K, dtype=np.float32)
    V = np.asarray(V, dtype=np.float32)
    Wq = np.asarray(Wq, dtype=np.float32)
    Wk = np.asarray(Wk, dtype=np.float32)
    Wo = np.asarray(Wo, dtype=np.float32)
    bq = np.ascontiguousarray(np.asarray(bq, dtype=np.float32))
    bk = np.ascontiguousarray(np.asarray(bk, dtype=np.float32))
    mask = np.asarray(key_padding_mask).astype(bool)

    keep = [np.flatnonzero(~mask[b]) for b in range(B)]
    tkp = max(128, ((max(len(ix) for ix in keep) + 127) // 128) * 128)

    wqT = np.ascontiguousarray(Wq.T.astype(NP_BF16))
    wkT = np.ascontiguousarray(Wk.T.astype(NP_BF16))
    woT = np.ascontiguousarray(Wo.T.astype(NP_BF16))
    ones_col = np.ones((128, 1), NP_BF16)
    ones_row = np.ones((1, 128), NP_BF16)
    qTb, kTb, vTb, mbb = [], [], [], []
    for b in range(B):
        ix = keep[b]
        qTb.append(np.ascontiguousarray(Q[b].T.astype(NP_BF16)))
        kt = np.zeros((D, tkp), NP_BF16)
        kt[:, : len(ix)] = K[b].T[:, ix].astype(NP_BF16)
        kTb.append(kt)
        vt = np.zeros((D, tkp), NP_BF16)
        vt[:, : len(ix)] = V[b].T[:, ix].astype(NP_BF16)
        # pre-arrange k-sliced: [p, kt*DT*128 + dt*128 + kk] = vT[dt*128+p, kt*128+kk]
        vTb.append(np.ascontiguousarray(
            vt.reshape(DT, 128, tkp // 128, 128).transpose(1, 2, 0, 3)
            .reshape(128, DT * tkp)))
        m = np.full(tkp, np.float32(MASK_BIAS), np.float32)
        m[: len(ix)] = 0.0
        mbb.append(np.ascontiguousarray(m.reshape(tkp // 128, 128).T))

    in_maps = []
    for c in range(NCORES):
        b, hg = divmod(c, NHG)
        in_maps.append(
            {
                "qT": qTb[b],
                "kT": kTb[b],
                "vT": vTb[b],
                "wqT": np.ascontiguousarray(wqT[:, hg * 64 : (hg + 1) * 64]),
                "wkT": np.ascontiguousarray(wkT[:, hg * 64 : (hg + 1) * 64]),
                "woT": np.ascontiguousarray(
                    woT[hg * HPC * DV : (hg + 1) * HPC * DV, :]
                    .reshape(HPC * DT, 128, O).transpose(1, 0, 2)
                    .reshape(128, HPC * DT * O)
                ),
                "bq": np.ascontiguousarray(bq[hg * 64 : (hg + 1) * 64]),
                "bk": np.ascontiguousarray(bk[hg * 64 : (hg + 1) * 64]),
                "mb": mbb[b],
                "ones_d": ones_col,
                "ones_row_d": ones_row,
            }
        )
    return tkp, in_maps


def run(inputs: dict, **spmd_kwargs):
    tkp, in_maps = _prep_in_maps(**inputs)
    nc = _get_nc(tkp)
    res = run_bass_kernel_spmd(nc, in_maps, core_ids=list(range(NCORES)),
                               **spmd_kwargs)
    bo = np.ascontiguousarray(np.asarray(inputs["bo"], dtype=np.float32))
    out = np.empty((B, TQ, O), np.float32)
    for b in range(B):
        acc = res.results[b * NHG]["out"]
        for hg in range(1, NHG):
            acc = acc + res.results[b * NHG + hg]["out"]
        out[b] = acc.T + bo
    return out, res


def kernel(**inputs) -> np.ndarray:
    out, _ = run(inputs)
    return out



# revision 2
# speedup vs baseline: 1.3094x; 1.3094x over previous
"""Low-dim-QK multi-head attention TRN2 Bass kernel v3 (8 NeuronCores).

v3 = v2 (Wtil refactor: out = sum_h A_h @ (V @ Wo_h.T), (batch, head-group)
sharding, bf16 operands, host-side partial-sum reduction) plus the per-matmul
overhead attack:

- walrus `--enable-ldw-opt=true` (redundant load-weight elimination; safe
  here: every stationary operand is bf16 -- the known breakage is fp32/fp32r
  standalone-LDW pairing).
- Loop orders put identical stationary operands on consecutive matmuls:
    Wtil:  lhsT = vT[dt,kt] reused 4x over (h, ob)
    apply: lhsT = Wtil[h,kt,ob] reused 4x over q-chunks (out.T layout)
    scores: lhsT = klT[h,kt] reused 4x over q-chunks (as in v2)
- Denominators via ones-column stationary (1-column LDW is ~free):
  D[1, q] accumulated over kt per 512-chunk, reciprocal, then broadcast to
  rep[128, q] with a ones-row matmul; P is pre-normalized in place on
  VectorE.  Apply chains then accumulate both heads in one PSUM chain (18
  matmuls) and drain with a plain ScalarE copy.
- Output is written as out.T (1024, 2048); the host transposes during the
  partial-sum reduction.
"""

import numpy as np
import ml_dtypes

import concourse.bass as bass
import concourse.mybir as mybir
import concourse.tile as tile
from concourse import bacc
import concourse.bass_utils as _bass_utils
from concourse.bass_utils import run_bass_kernel_spmd

F32 = mybir.dt.float32
BF16 = mybir.dt.bfloat16
AF = mybir.ActivationFunctionType
NP_BF16 = ml_dtypes.bfloat16

# --- redundant-LDW elimination (bass-level post-pass) ---------------------
# The tile scheduler splits every matmul into an explicit Ldweights+Matmult
# pair.  When consecutive matmuls on the PE stream share an identical
# stationary operand, the re-load is redundant: the PE array still holds the
# weights.  walrus' own `--enable-ldw-opt` pass crashes on pre-split pairs,
# so dedup here instead, before nc.compile().  A dropped re-load's RAW wait
# is subsumed by the kept (identical) load's wait - semaphores are monotonic
# counters - but any waits/updates are conservatively moved to the following
# matmul anyway (generate_event_semaphores later legalizes multi-waits).


def _ldw_key(inst):
    ap = inst.ins[0]
    return (
        str(getattr(ap, "memref", None) or getattr(ap, "memsetref", "?")),
        getattr(ap, "offset", None),
        str(getattr(ap, "ap", None)),
        str(getattr(ap, "dtype", None)),
        tuple(inst.tile_position or ()) if getattr(inst, "tile_position", None) is not None else None,
        tuple(inst.tile_size or ()) if getattr(inst, "tile_size", None) is not None else None,
    )


def _dedup_ldweights(nc):
    removed = 0
    for fn in nc.m.functions:
        for blk in fn.blocks:
            insts = blk.instructions
            last_key = None
            keep = []
            i = 0
            n = len(insts)
            while i < n:
                inst = insts[i]
                if getattr(inst, "engine", None) != mybir.EngineType.PE:
                    keep.append(inst)
                    i += 1
                    continue
                op = type(inst).__name__
                if op == "InstLdweights" and getattr(inst, "perf_mode", None) is None:
                    key = _ldw_key(inst)
                    if key == last_key and key is not None:
                        # find the following PE instruction (its matmul) to
                        # inherit this load's waits/updates
                        si = inst.sync_info
                        if si is not None and (si.on_wait or si.on_update):
                            j = i + 1
                            tgt = None
                            while j < n:
                                if getattr(insts[j], "engine", None) == mybir.EngineType.PE:
                                    tgt = insts[j]
                                    break
                                j += 1
                            if tgt is None:
                                keep.append(inst)
                                last_key = key
                                i += 1
                                continue
                            tsi = tgt.sync_info
                            if tsi is None:
                                tgt.sync_info = si
                            else:
                                tsi.on_wait.extend(si.on_wait)
                                tsi.on_update.extend(si.on_update)
                        removed += 1
                        i += 1
                        continue
                    last_key = key
                    keep.append(inst)
                    i += 1
                    continue
                if op == "InstMatmult":
                    keep.append(inst)
                    i += 1
                    continue
                # any other PE instruction invalidates the loaded weights
                last_key = None
                keep.append(inst)
                i += 1
            if len(keep) != n:
                try:
                    insts[:] = keep
                except TypeError:
                    blk.instructions = keep
    return removed
# --------------------------------------------------------------------------

# Note: LDW dedup is numerically UNSAFE on this hardware - walrus pairs each
# MMUL with its LDW via auto-flipping double-buffered weights, so an MMUL
# without its own LDW flips to a stale buffer (verified: rel-err 0.27 with
# dedup, 4.3e-3 without; runtime identical, i.e. the loads were already
# hidden).  Keep False.
DEDUP_LDW = False

B = 2
TQ = 2048
TK = 2048
D = 1024
H = 8
HD = 32
DV = 1024
O = 1024
NCORES = 8
NHG = 4
HPC = H // NHG    # 2 heads per core
DT = D // 128
QB = TQ // 128
SCALE = 1.0 / float(np.sqrt(HD))
MASK_BIAS = -60.0


def _chunks(n, c):
    out = []
    k0 = 0
    while k0 < n:
        out.append((k0, min(c, n - k0)))
        k0 += c
    return out


def _body(nc, t, pools, tkp, ablate=frozenset()):
    qT, kT, vT, wqT, wkT, woT, bq, bk, mb, ones_d, ones_row_d, out = t
    const, stream, stage, ps, ps_d = pools
    KT = tkp // 128

    # ---- persistent SBUF tiles --------------------------------------
    vT_sb = const.tile([128, tkp // 128, DT * 128], BF16, tag="vt")
    woT_sb = const.tile([128, HPC * DT, O], BF16, tag="wo")
    qlT = const.tile([64, TQ], BF16, tag="qlT")
    klT = const.tile([64, tkp], BF16, tag="klT")
    p_sb = [
        const.tile([128, KT, TQ], BF16, tag=f"p{h}", name=f"p{h}")
        for h in range(HPC)
    ]
    w_sb = const.tile([128, HPC * KT, O], BF16, tag="wt")
    dinv = const.tile([1, HPC, TQ], BF16, tag="dinv")
    rep_sb = const.tile([128, HPC, TQ], BF16, tag="rep")
    mb_sb = const.tile([128, KT], F32, tag="mb")
    bq_sb = const.tile([64, 1], F32, tag="bq")
    bk_sb = const.tile([64, 1], F32, tag="bk")
    wq_sb = const.tile([128, DT, 64], BF16, tag="wq")
    wk_sb = const.tile([128, DT, 64], BF16, tag="wk")
    ones_col = const.tile([128, 1], BF16, tag="onc")
    ones_row = const.tile([1, 128], BF16, tag="onr")

    # ---- phase A: projections ---------------------------------------
    # Wtil feeds (6.25 MB) go on the Activation HWDGE queue so they stream in
    # parallel with the SP queue's projection feeds; both overlap proj+scores
    # compute.  Issue first: queue order only matters within a queue.
    if "bigdma" not in ablate:
        for wc in range(DT):
            nc.scalar.dma_start(
                out=woT_sb[:, wc, :],
                in_=woT.ap()[:, wc * O : (wc + 1) * O],
            )
        for kt_ in range(KT):
            nc.scalar.dma_start(
                out=vT_sb[:, kt_, :],
                in_=vT.ap()[:, kt_ * DT * 128 : (kt_ + 1) * DT * 128],
            )
        for wc in range(DT, HPC * DT):
            nc.scalar.dma_start(
                out=woT_sb[:, wc, :],
                in_=woT.ap()[:, wc * O : (wc + 1) * O],
            )
    nc.sync.dma_start(out=wq_sb, in_=wqT.ap().rearrange("(dt p) r -> p dt r", p=128))
    nc.sync.dma_start(out=bq_sb, in_=bq.ap().rearrange("(g p) -> p g", p=64))

    ql_ps = [ps.tile([128, 512], F32, tag="ps", name=f"qlps{i}") for i in range(2)]
    for dt_ in range(DT):
        qc = stream.tile([128, TQ], BF16, tag="qs", name=f"qc{dt_}")
        nc.sync.dma_start(out=qc, in_=qT.ap()[dt_ * 128 : (dt_ + 1) * 128, :])
        if dt_ == 0:
            nc.sync.dma_start(
                out=wk_sb, in_=wkT.ap().rearrange("(dt p) r -> p dt r", p=128)
            )
            nc.sync.dma_start(out=bk_sb, in_=bk.ap().rearrange("(g p) -> p g", p=64))
            nc.sync.dma_start(out=mb_sb, in_=mb.ap())
            nc.sync.dma_start(out=ones_col, in_=ones_d.ap())
            nc.sync.dma_start(out=ones_row, in_=ones_row_d.ap())
        for qc4 in range(4):
            nc.tensor.matmul(
                ql_ps[qc4 // 2][qc4 % 2 * 64 : qc4 % 2 * 64 + 64, :],
                lhsT=wq_sb[:, dt_, :],
                rhs=qc[:, qc4 * 512 : (qc4 + 1) * 512],
                start=(dt_ == 0),
                stop=(dt_ == DT - 1),
            )
    for qc4 in range(4):
        nc.scalar.activation(
            out=qlT[:, qc4 * 512 : (qc4 + 1) * 512],
            in_=ql_ps[qc4 // 2][qc4 % 2 * 64 : qc4 % 2 * 64 + 64, :],
            func=AF.Identity, bias=bq_sb, scale=1.0,
        )

    kchunks = _chunks(tkp, 512)
    kl_ps = [
        ps.tile([128, 512], F32, tag="ps", name=f"klps{i}")
        for i in range((len(kchunks) + 1) // 2)
    ]
    for dt_ in range(DT):
        kc = stream.tile([128, tkp], BF16, tag="ks", name=f"kc{dt_}")
        nc.sync.dma_start(out=kc, in_=kT.ap()[dt_ * 128 : (dt_ + 1) * 128, :])
        for ci, (k0, csz) in enumerate(kchunks):
            nc.tensor.matmul(
                kl_ps[ci // 2][ci % 2 * 64 : ci % 2 * 64 + 64, 0:csz],
                lhsT=wk_sb[:, dt_, :],
                rhs=kc[:, k0 : k0 + csz],
                start=(dt_ == 0),
                stop=(dt_ == DT - 1),
            )
    for ci, (k0, csz) in enumerate(kchunks):
        nc.scalar.activation(
            out=klT[:, k0 : k0 + csz],
            in_=kl_ps[ci // 2][ci % 2 * 64 : ci % 2 * 64 + 64, 0:csz],
            func=AF.Identity, bias=bk_sb, scale=1.0,
        )

    # ---- phase B: scores (heads paired on PE row-strips) ------------
    def emit_scores():
        for kt_ in range(KT):
            for h in range(HPC):
                for qc4 in range(4):
                    st = ps.tile(
                        [128, 512], F32, tag="ps", name=f"st{h}_{kt_}_{qc4}"
                    )
                    nc.tensor.matmul(
                        st,
                        lhsT=klT[h * 32 : h * 32 + 32, kt_ * 128 : (kt_ + 1) * 128],
                        rhs=qlT[h * 32 : h * 32 + 32, qc4 * 512 : (qc4 + 1) * 512],
                        start=True,
                        stop=True,
                    )
                    nc.scalar.activation(
                        out=p_sb[h][:, kt_, qc4 * 512 : (qc4 + 1) * 512],
                        in_=st, func=AF.Exp,
                        bias=mb_sb[:, kt_ : kt_ + 1], scale=SCALE,
                    )

    # ---- phase C: Wtil per (h, kt); h0 only needs woT groups 0..DT-1 and
    # vT k-slice kt, so chains start as soon as those DMAs land -----------
    def emit_wtil_h_kt(h, kt_):
        wps = [
            ps.tile([128, 512], F32, tag="ps", name=f"wp{h}_{kt_}_{i}")
            for i in range(2)
        ]
        for dt_ in range(DT):
            for ob in range(2):
                nc.tensor.matmul(
                    wps[ob],
                    lhsT=vT_sb[:, kt_, dt_ * 128 : (dt_ + 1) * 128],
                    rhs=woT_sb[:, h * DT + dt_, ob * 512 : (ob + 1) * 512],
                    start=(dt_ == 0),
                    stop=(dt_ == DT - 1),
                )
        for ob in range(2):
            nc.vector.tensor_copy(
                out=w_sb[:, h * KT + kt_, ob * 512 : (ob + 1) * 512],
                in_=wps[ob],
            )

    # ---- phase D: denominators + P normalization --------------------
    def emit_denom(h):
        for qc4 in range(4):
            dps = ps_d.tile([1, 512], F32, tag="pd", name=f"dps{h}_{qc4}")
            for kt_ in range(KT):
                nc.tensor.matmul(
                    dps,
                    lhsT=ones_col,
                    rhs=p_sb[h][:, kt_, qc4 * 512 : (qc4 + 1) * 512],
                    start=(kt_ == 0),
                    stop=(kt_ == KT - 1),
                )
            nc.vector.reciprocal(
                out=dinv[:, h, qc4 * 512 : (qc4 + 1) * 512], in_=dps
            )
        for qc4 in range(4):
            rps = ps.tile([128, 512], F32, tag="ps", name=f"rps{h}_{qc4}")
            nc.tensor.matmul(
                rps,
                lhsT=ones_row,
                rhs=dinv[:, h, qc4 * 512 : (qc4 + 1) * 512],
                start=True,
                stop=True,
            )
            nc.vector.tensor_copy(
                out=rep_sb[:, h, qc4 * 512 : (qc4 + 1) * 512], in_=rps
            )
        for kt_ in range(KT):
            nc.vector.tensor_mul(
                p_sb[h][:, kt_, :], p_sb[h][:, kt_, :], rep_sb[:, h, :]
            )

    if "scores" in ablate:
        for h in range(HPC):
            nc.vector.memset(p_sb[h][:, 0, 0:512], 1.0)
    if "wtil" in ablate:
        nc.vector.memset(w_sb[:, 0, 0:512], 1.0)
    if "scores" not in ablate:
        emit_scores()
    # Interleave denominator/normalization per head into the Wtil stream so
    # P-hat for both heads is ready the moment Wtil finishes (no PE bubble
    # before apply).  D_h is emitted once exp_h has certainly drained
    # (ScalarE exp of head h completes well inside the first few Wtil
    # kt-blocks' PE time).
    for h in range(HPC):
        for kt_ in range(KT):
            if "wtil" not in ablate:
                emit_wtil_h_kt(h, kt_)
            if kt_ == min(4, KT - 1) and "denom" not in ablate:
                emit_denom(h)

    # ---- phase E: apply, stationary Wtil[h,kt,ob] reused over qc ----
    if "apply" in ablate:
        return
    out_r = out.ap().rearrange("(ob p) q -> p ob q", p=128)
    for ob in range(8):
        uacc = [
            ps.tile([128, 512], F32, tag="ps", name=f"ua{ob}_{qc4}")
            for qc4 in range(4)
        ]
        for h in range(HPC):
            for kt_ in range(KT):
                for qc4 in range(4):
                    nc.tensor.matmul(
                        uacc[qc4],
                        lhsT=w_sb[:, h * KT + kt_, ob * 128 : (ob + 1) * 128],
                        rhs=p_sb[h][:, kt_, qc4 * 512 : (qc4 + 1) * 512],
                        start=(h == 0 and kt_ == 0),
                        stop=(h == HPC - 1 and kt_ == KT - 1),
                    )
        for qc4 in range(4):
            ot = stage.tile([128, 512], F32, tag="ot", name=f"ot{ob}_{qc4}")
            nc.scalar.copy(out=ot, in_=uacc[qc4])
            nc.sync.dma_start(
                out=out_r[:, ob, qc4 * 512 : (qc4 + 1) * 512], in_=ot
            )


def _build_kernel(nc, t, tkp, loop_n=None, ablate=frozenset()):
    tc = tile.TileContext(nc)
    with tc, nc.allow_low_precision(
        reason="bf16 matmul operands; all accumulation is fp32 in PSUM"
    ):
        pools = (
            tc.alloc_tile_pool(name="const", bufs=1),
            tc.alloc_tile_pool(name="stream", bufs=2),
            tc.alloc_tile_pool(name="stage", bufs=2),
            tc.alloc_tile_pool(name="ps", bufs=6, space="PSUM"),
            tc.alloc_tile_pool(name="ps_d", bufs=2, space="PSUM"),
        )
        if loop_n is None:
            _body(nc, t, pools, tkp, ablate=ablate)
        else:
            with tc.For_i(0, loop_n, 1, staggered_reset=True):
                _body(nc, t, pools, tkp, ablate=ablate)
        for p in reversed(pools):
            p.release()
    if DEDUP_LDW:
        _dedup_ldweights(nc)
    nc.compile()


def _declare_io(nc, tkp):
    qT = nc.dram_tensor("qT", (D, TQ), BF16, kind="ExternalInput")
    kT = nc.dram_tensor("kT", (D, tkp), BF16, kind="ExternalInput")
    vT = nc.dram_tensor("vT", (128, DT * tkp), BF16, kind="ExternalInput")  # k-sliced
    wqT = nc.dram_tensor("wqT", (D, 64), BF16, kind="ExternalInput")
    wkT = nc.dram_tensor("wkT", (D, 64), BF16, kind="ExternalInput")
    woT = nc.dram_tensor("woT", (128, HPC * DT * O), BF16, kind="ExternalInput")
    bq = nc.dram_tensor("bq", (64,), F32, kind="ExternalInput")
    bk = nc.dram_tensor("bk", (64,), F32, kind="ExternalInput")
    mb = nc.dram_tensor("mb", (128, tkp // 128), F32, kind="ExternalInput")
    ones_d = nc.dram_tensor("ones_d", (128, 1), BF16, kind="ExternalInput")
    ones_row_d = nc.dram_tensor("ones_row_d", (1, 128), BF16, kind="ExternalInput")
    out = nc.dram_tensor("out", (O, TQ), F32, kind="ExternalOutput")
    return (qT, kT, vT, wqT, wkT, woT, bq, bk, mb, ones_d, ones_row_d, out)


def build_nc(tkp=1152, loop_n=None, ablate=frozenset()):
    nc = bacc.Bacc("TRN2", target_bir_lowering=False, debug=False,
                   num_devices=NCORES)
    t = _declare_io(nc, tkp)
    _build_kernel(nc, t, tkp, loop_n=loop_n, ablate=ablate)
    return nc


_NC_CACHE = {}


def _get_nc(tkp):
    if tkp not in _NC_CACHE:
        _NC_CACHE[tkp] = build_nc(tkp)
    return _NC_CACHE[tkp]


def _prep_in_maps(Q, K, V, Wq, bq, Wk, bk, Wo, bo, key_padding_mask):
    Q = np.asarray(Q, dtype=np.float32)
    K = np.asarray(K, dtype=np.float32)
    V = np.asarray(V, dtype=np.float32)
    Wq = np.asarray(Wq, dtype=np.float32)
    Wk = np.asarray(Wk, dtype=np.float32)
    Wo = np.asarray(Wo, dtype=np.float32)
    bq = np.ascontiguousarray(np.asarray(bq, dtype=np.float32))
    bk = np.ascontiguousarray(np.asarray(bk, dtype=np.float32))
    mask = np.asarray(key_padding_mask).astype(bool)

    keep = [np.flatnonzero(~mask[b]) for b in range(B)]
    tkp = max(128, ((max(len(ix) for ix in keep) + 127) // 128) * 128)

    wqT = np.ascontiguousarray(Wq.T.astype(NP_BF16))
    wkT = np.ascontiguousarray(Wk.T.astype(NP_BF16))
    woT = np.ascontiguousarray(Wo.T.astype(NP_BF16))
    ones_col = np.ones((128, 1), NP_BF16)
    ones_row = np.ones((1, 128), NP_BF16)
    qTb, kTb, vTb, mbb = [], [], [], []
    for b in range(B):
        ix = keep[b]
        qTb.append(np.ascontiguousarray(Q[b].T.astype(NP_BF16)))
        kt = np.zeros((D, tkp), NP_BF16)
        kt[:, : len(ix)] = K[b].T[:, ix].astype(NP_BF16)
        kTb.append(kt)
        vt = np.zeros((D, tkp), NP_BF16)
        vt[:, : len(ix)] = V[b].T[:, ix].astype(NP_BF16)
        # pre-arrange k-sliced: [p, kt*DT*128 + dt*128 + kk] = vT[dt*128+p, kt*128+kk]
        vTb.append(np.ascontiguousarray(
            vt.reshape(DT, 128, tkp // 128, 128).transpose(1, 2, 0, 3)
            .reshape(128, DT * tkp)))
        m = np.full(tkp, np.float32(MASK_BIAS), np.float32)
        m[: len(ix)] = 0.0
        mbb.append(np.ascontiguousarray(m.reshape(tkp // 128, 128).T))

    in_maps = []
    for c in range(NCORES):
        b, hg = divmod(c, NHG)
        in_maps.append(
            {
                "qT": qTb[b],
                "kT": kTb[b],
                "vT": vTb[b],
                "wqT": np.ascontiguousarray(wqT[:, hg * 64 : (hg + 1) * 64]),
                "wkT": np.ascontiguousarray(wkT[:, hg * 64 : (hg + 1) * 64]),
                "woT": np.ascontiguousarray(
                    woT[hg * HPC * DV : (hg + 1) * HPC * DV, :]
                    .reshape(HPC * DT, 128, O).transpose(1, 0, 2)
                    .reshape(128, HPC * DT * O)
                ),
                "bq": np.ascontiguousarray(bq[hg * 64 : (hg + 1) * 64]),
                "bk": np.ascontiguousarray(bk[hg * 64 : (hg + 1) * 64]),
                "mb": mbb[b],
                "ones_d": ones_col,
                "ones_row_d": ones_row,
            }
        )
    return tkp, in_maps


def run(inputs: dict, **spmd_kwargs):
    tkp, in_maps = _prep_in_maps(**inputs)
    nc = _get_nc(tkp)
    res = run_bass_kernel_spmd(nc, in_maps, core_ids=list(range(NCORES)),
                               **spmd_kwargs)
    bo = np.ascontiguousarray(np.asarray(inputs["bo"], dtype=np.float32))
    out = np.empty((B, TQ, O), np.float32)
    for b in range(B):
        acc = res.results[b * NHG]["out"]
        for hg in range(1, NHG):
            acc = acc + res.results[b * NHG + hg]["out"]
        out[b] = acc.T + bo
    return out, res


def kernel(**inputs) -> np.ndarray:
    out, _ = run(inputs)
    return out

